# revision 1
# baseline (speedup 1.0000x reference)
"""Self-attention (8 heads, d=64, B=2, N=4096, D=512) on 8 TRN2 NeuronCores.

Sharding: batch*heads across cores — core c handles batch b=c//4, heads
(2*(c%4), 2*(c%4)+1). Projection weights are sliced per-core on the host;
x is pre-transposed on the host so the device needs no transposes at all.

Device dataflow (per core, fully transposed "scoresT" formulation):
  qT2/kT2 [hd=128, n]  = W.T-chunks @ xT-chunks          (PE, f32r)
  v2      [n, hd+ones] natural                            (PE, bf16 store)
  per head h, per q-chunk qq (1024 wide):
    for kc in 32:  scT psum[128k,1024q] = kh.T @ qh       (PE)
                   attnT = exp(scT*SCALE)  -> bf16 SBUF   (ACT, scale fused)
                   av[65,1024] += v2'[kc].T @ attnT       (PE, accumulate)
    row 64 of av = softmax denominator (ones column of v2')
    outT[h] = av[:64] * (1/denom)                         (DVE + DMA bcast)
  partial[n,512] = sum_h outT[h].T @ woT[h]               (PE)
Host: out[b] = sum of its 4 cores' partials + bo.
"""
import numpy as np
import ml_dtypes
from contextlib import ExitStack

import concourse.bass as bass
from concourse import bacc
import concourse.mybir as mybir
import concourse.tile as tile
from concourse.bass_utils import run_bass_kernel_spmd

B, N, D = 2, 4096, 512
HEADS, DH = 8, 64
SCALE = DH ** -0.5

F32 = mybir.dt.float32
F32R = mybir.dt.bfloat16  # matmul operand dtype (bf16: 1cyc/row, standard path)
BF16 = mybir.dt.bfloat16

QQ_W = 1024          # q-chunk width in the attention loop
N_QQ = N // QQ_W     # 4
N_KC = N // 128      # 32 key chunks
DCH = D // 128       # 4 contraction chunks for projections


def build_bass():
    nc = bacc.Bacc(None, target_bir_lowering=False)

    xT = nc.dram_tensor("xT", [D, N], F32R, kind="ExternalInput")
    wqT = nc.dram_tensor("wqT", [D, 128], F32R, kind="ExternalInput")
    wkT = nc.dram_tensor("wkT", [D, 128], F32R, kind="ExternalInput")
    wvT = nc.dram_tensor("wvT", [D, 128], F32R, kind="ExternalInput")
    woT = nc.dram_tensor("woT", [2, 64, D], F32R, kind="ExternalInput")
    out = nc.dram_tensor("out", [N, D], F32, kind="ExternalOutput")
    recip_dram = nc.dram_tensor("recip_scratch", [N_QQ, 2, QQ_W], F32)

    with tile.TileContext(nc) as tc, ExitStack() as ctx:
        const = ctx.enter_context(tc.tile_pool(name="const", bufs=1))

        # ---- load inputs ----
        xT_sb = const.tile([128, DCH, N], F32R)            # xT[(c p), n] -> [p, c, n]
        nc.sync.dma_start(out=xT_sb, in_=xT.rearrange("(c p) n -> p c n", p=128))
        wq_sb = const.tile([128, DCH, 128], F32R)
        nc.sync.dma_start(out=wq_sb, in_=wqT.rearrange("(c p) m -> p c m", p=128))
        wk_sb = const.tile([128, DCH, 128], F32R)
        nc.sync.dma_start(out=wk_sb, in_=wkT.rearrange("(c p) m -> p c m", p=128))
        wv_sb = const.tile([128, DCH, 128], F32R)
        nc.sync.dma_start(out=wv_sb, in_=wvT.rearrange("(c p) m -> p c m", p=128))
        wo_sb = const.tile([64, 2, D], F32R)
        nc.sync.dma_start(out=wo_sb, in_=woT.rearrange("h d n -> d h n"))

        qT2 = const.tile([128, N], F32R)                   # [2-head d, n]
        kT2 = const.tile([128, N], F32R)
        v2 = const.tile([128, N_KC, 130], BF16)            # [k-part, kc, (v_h0|1|v_h1|1)]
        outT = const.tile([64, 2, N], F32R)                # normalized per-head av

        # ---- projections ----
        with tc.tile_pool(name="proj_psum", bufs=3, space="PSUM") as proj_psum:
            for nt in range(N // 512):
                pq = proj_psum.tile([128, 512], F32, tag="pj")
                for c in range(DCH):
                    nc.tensor.matmul(pq, wq_sb[:, c, :], xT_sb[:, c, bass.ts(nt, 512)],
                                     start=(c == 0), stop=(c == DCH - 1))
                nc.vector.tensor_copy(qT2[:, bass.ts(nt, 512)], pq)
            for nt in range(N // 512):
                pk = proj_psum.tile([128, 512], F32, tag="pj")
                for c in range(DCH):
                    nc.tensor.matmul(pk, wk_sb[:, c, :], xT_sb[:, c, bass.ts(nt, 512)],
                                     start=(c == 0), stop=(c == DCH - 1))
                nc.vector.tensor_copy(kT2[:, bass.ts(nt, 512)], pk)
            # v natural: out[n-tile, hd] = xT-chunk.T @ wv-chunk
            for kc in range(N_KC):
                pv = proj_psum.tile([128, 128], F32, tag="pv")
                for c in range(DCH):
                    nc.tensor.matmul(pv, xT_sb[:, c, bass.ts(kc, 128)], wv_sb[:, c, :],
                                     start=(c == 0), stop=(c == DCH - 1))
                # interleave the two heads' 64-col halves into v2 (cols 0-63, 65-128)
                nc.vector.tensor_copy(v2[:, kc, 0:64], pv[:, 0:64])
                nc.vector.tensor_copy(v2[:, kc, 65:129], pv[:, 64:128])
        # ones columns for the softmax-denominator trick
        nc.vector.memset(v2[:, :, 64], 1.0)
        nc.vector.memset(v2[:, :, 129], 1.0)

        # ---- attention ----
        with (
            tc.tile_pool(name="sc_psum", bufs=2, space="PSUM") as sc_psum,
            tc.tile_pool(name="av_psum", bufs=2, space="PSUM") as av_psum,
            tc.tile_pool(name="attn_sb", bufs=4) as attn_sb,
            tc.tile_pool(name="norm_sb", bufs=2) as norm_sb,
        ):
            for qq in range(N_QQ):
                avs = []
                for h in range(2):
                    av = av_psum.tile([65, QQ_W], F32, tag="av", name=f"av_{qq}_{h}")
                    avs.append(av)
                for kc in range(N_KC):
                    for h in range(2):
                        sc = sc_psum.tile([128, QQ_W], F32, tag="sc", name=f"sc_{qq}_{kc}_{h}")
                        for s in range(QQ_W // 512):
                            nc.tensor.matmul(
                                sc[:, bass.ts(s, 512)],
                                kT2[h * 64:(h + 1) * 64, bass.ts(kc, 128)],
                                qT2[h * 64:(h + 1) * 64, qq * QQ_W + s * 512:qq * QQ_W + (s + 1) * 512],
                                start=True, stop=True)
                        at = attn_sb.tile([128, QQ_W], BF16, tag="at", name=f"at_{qq}_{kc}_{h}")
                        nc.scalar.activation(at, sc, mybir.ActivationFunctionType.Exp,
                                             scale=float(SCALE))
                        for s in range(QQ_W // 512):
                            nc.tensor.matmul(
                                avs[h][:, bass.ts(s, 512)],
                                v2[:, kc, h * 65:(h + 1) * 65],
                                at[:, bass.ts(s, 512)],
                                start=(kc == 0), stop=(kc == N_KC - 1))
                # normalize: outT[h][:, qq] = av[:64] * 1/av[64]
                for h in range(2):
                    av = avs[h]
                    rc = norm_sb.tile([128, QQ_W], F32, tag="rc", name=f"rc_{qq}_{h}")
                    nc.vector.reciprocal(rc[64:65, :], av[64:65, :])
                    bc = norm_sb.tile([64, QQ_W], F32, tag="bc", name=f"bc_{qq}_{h}")
                    nc.sync.dma_start(out=recip_dram[qq:qq + 1, h, :], in_=rc[64:65, :])
                    src = recip_dram[qq, h, :]
                    bcast = bass.AP(tensor=src.tensor, offset=src.offset,
                                    ap=[[0, 64]] + src.ap)
                    nc.sync.dma_start(out=bc, in_=bcast)
                    nc.vector.tensor_mul(outT[:, h, qq * QQ_W:(qq + 1) * QQ_W], av[0:64, :], bc)

        # ---- output projection ----
        with (
            tc.tile_pool(name="op_psum", bufs=3, space="PSUM") as op_psum,
            tc.tile_pool(name="op_sb", bufs=3) as op_sb,
        ):
            for nt in range(N // 128):
                po = op_psum.tile([128, D], F32, tag="po")
                nc.tensor.matmul(po, outT[:, 0, bass.ts(nt, 128)], wo_sb[:, 0, :],
                                 start=True, stop=False)
                nc.tensor.matmul(po, outT[:, 1, bass.ts(nt, 128)], wo_sb[:, 1, :],
                                 start=False, stop=True)
                ob = op_sb.tile([128, D], F32, tag="ob")
                nc.vector.tensor_copy(ob, po)
                nc.sync.dma_start(out=out[bass.ts(nt, 128), :], in_=ob)

    nc.compile()
    return nc


_NC_CACHE = None


def build_in_maps(x, Wq, Wk, Wv, Wo):
    bf = ml_dtypes.bfloat16
    x = np.asarray(x, np.float32)
    Wq, Wk, Wv, Wo = (np.asarray(a, np.float32) for a in (Wq, Wk, Wv, Wo))
    in_maps = []
    for c in range(8):
        b = c // 4
        h0 = 2 * (c % 4)
        xT = np.ascontiguousarray(x[b].T.astype(bf))
        wqT = np.ascontiguousarray(Wq[h0 * 64:(h0 + 2) * 64].T.astype(bf))
        wkT = np.ascontiguousarray(Wk[h0 * 64:(h0 + 2) * 64].T.astype(bf))
        wvT = np.ascontiguousarray(Wv[h0 * 64:(h0 + 2) * 64].T.astype(bf))
        woT = np.stack([np.ascontiguousarray(Wo[:, (h0 + h) * 64:(h0 + h + 1) * 64].T.astype(bf))
                        for h in range(2)])
        in_maps.append({"xT": xT, "wqT": wqT, "wkT": wkT, "wvT": wvT, "woT": woT})
    return in_maps


def kernel(x, Wq, Wk, Wv, Wo, bo):
    global _NC_CACHE
    bo = np.asarray(bo, np.float32)
    in_maps = build_in_maps(x, Wq, Wk, Wv, Wo)

    if _NC_CACHE is None:
        _NC_CACHE = build_bass()
    res = run_bass_kernel_spmd(_NC_CACHE, in_maps, list(range(8)))
    partials = [np.asarray(res.results[c]["out"], np.float32) for c in range(8)]

    out = np.empty((B, N, D), np.float32)
    for b in range(B):
        out[b] = partials[4 * b] + partials[4 * b + 1] + partials[4 * b + 2] + partials[4 * b + 3] + bo
    return out


if __name__ == "__main__":
    nc = build_bass()
    print("built ok")



# revision 7
# speedup vs baseline: 5911.3779x; 5911.3779x over previous
"""Self-attention (8 heads, d=64, B=2, N=4096, D=512) on 8 TRN2 NeuronCores.

Sharding: batch*heads across cores — core c handles batch b=c//4, heads
(2*(c%4), 2*(c%4)+1). Projection weights are sliced per-core on the host;
x is pre-transposed on the host so the device needs no transposes at all.

v2: software-pipelined attention loop with the softmax exp split across
BOTH the Scalar (ACT) and Vector (DVE) engines:
  - ACT computes exp(sc*SCALE) for one 512-wide half of each score tile
    (hardware spline, exact).
  - DVE computes the other half with a Schraudolph-style bit-trick:
    bf16_bits(e^x) ~= int16(x * 128*log2e*SCALE + 128*(127-0.0573)),
    emitted as one tensor_scalar (mult,add) with an int16-bitcast write
    into the bf16 attn tile (fp32->int16 conversion rounds-to-nearest).
  The halves alternate with kc parity so every query row mixes exact and
  approximated weights (rel err ~9e-3 vs 2e-2 budget).
Pipelined emission per kc: sc MMs (kc) -> exps (kc) -> av MMs (kc-1), so
the PE never idles waiting on the exp and the HAM clock-gate stays warm.

Device dataflow (per core, fully transposed "scoresT" formulation):
  qT2/kT2 [hd=128, n]  = W.T-chunks @ xT-chunks          (PE)
  v2      [n, hd+ones] natural                            (PE, bf16 store)
  per qq (1024 queries), kc (128 keys), h (2 heads):
    scT psum[128k,1024q] = kh.T @ qh   (interleaved h0/h1 -> row-group pairs)
    attnT = exp(scT*SCALE) -> bf16 SBUF   (ACT half | DVE half)
    av[65,1024] += v2'[kc].T @ attnT      (PE, accumulate; ones col = denom)
  drain av -> SBUF (ScalarE copy), reciprocal_approx_fast on denom row,
  DMA-broadcast, normalize mul -> outT (DVE)
  partial[n,512] = sum_h outT[h].T @ woT[h]               (PE)
Host: out[b] = sum of its 4 cores' partials + bo.
"""
import numpy as np
import ml_dtypes
from contextlib import ExitStack

import concourse.bass as bass
from concourse import bacc
import concourse.mybir as mybir
import concourse.tile as tile
from concourse.bass_utils import run_bass_kernel_spmd

B, N, D = 2, 4096, 512
HEADS, DH = 8, 64
SCALE = DH ** -0.5

F32 = mybir.dt.float32
F32R = mybir.dt.bfloat16  # matmul operand dtype (bf16: 1cyc/row)
BF16 = mybir.dt.bfloat16
I16 = mybir.dt.int16

QQ_W = 1024          # q-chunk width in the attention loop
N_QQ = N // QQ_W     # 4
N_KC = N // 128      # 32 key chunks
DCH = D // 128       # 4 contraction chunks for projections

LOG2E = 1.4426950408889634
A_SCH = float(128.0 * LOG2E * SCALE)          # fold attention scale in
B_SCH = float(128.0 * (127.0 - 0.057304959))  # equal-ripple bias

EXP_MODE = "split"   # 'split' = ACT half + DVE Schraudolph half; 'act' = all ACT
RECIP_MODE = "exact"  # 'exact' = nc.vector.reciprocal; 'approx' = custom DVE op


def build_bass():
    nc = bacc.Bacc(None, target_bir_lowering=False)

    xT = nc.dram_tensor("xT", [D, N], F32R, kind="ExternalInput")
    wqT = nc.dram_tensor("wqT", [D, 128], F32R, kind="ExternalInput")
    wkT = nc.dram_tensor("wkT", [D, 128], F32R, kind="ExternalInput")
    wvT = nc.dram_tensor("wvT", [D, 128], F32R, kind="ExternalInput")
    woT = nc.dram_tensor("woT", [2, 64, D], F32R, kind="ExternalInput")
    out = nc.dram_tensor("out", [N, D], F32, kind="ExternalOutput")
    recip_dram = nc.dram_tensor("recip_scratch", [N_QQ, 2, QQ_W], F32)
    denom_dram = nc.dram_tensor("denom_scratch", [N_QQ, 2, QQ_W], F32)

    with tile.TileContext(nc) as tc, ExitStack() as ctx:
        const = ctx.enter_context(tc.tile_pool(name="const", bufs=1))

        # ---- load inputs (xT chunked along n so projections start early) ----
        xT_sb = const.tile([128, DCH, N], F32R)            # xT[(c p), n] -> [p, c, n]
        xT_r = xT.rearrange("(c p) n -> p c n", p=128)
        for nch in range(4):
            nc.sync.dma_start(out=xT_sb[:, :, bass.ts(nch, N // 4)],
                              in_=xT_r[:, :, bass.ts(nch, N // 4)])
        wq_sb = const.tile([128, DCH, 128], F32R)
        nc.sync.dma_start(out=wq_sb, in_=wqT.rearrange("(c p) m -> p c m", p=128))
        wk_sb = const.tile([128, DCH, 128], F32R)
        nc.sync.dma_start(out=wk_sb, in_=wkT.rearrange("(c p) m -> p c m", p=128))
        wv_sb = const.tile([128, DCH, 128], F32R)
        nc.sync.dma_start(out=wv_sb, in_=wvT.rearrange("(c p) m -> p c m", p=128))
        wo_sb = const.tile([64, 2, D], F32R)
        nc.sync.dma_start(out=wo_sb, in_=woT.rearrange("h d n -> d h n"))

        qT2 = const.tile([128, N], F32R)                   # [2-head d, n]
        kT2 = const.tile([128, N], F32R)
        v2 = const.tile([128, N_KC, 130], BF16)            # [k-part, kc, (v_h0|1|v_h1|1)]
        outT = const.tile([64, 2, N], F32R)                # normalized per-head av

        # ---- projections ----
        with tc.tile_pool(name="proj_psum", bufs=3, space="PSUM") as proj_psum:
            for nt in range(N // 512):
                pk = proj_psum.tile([128, 512], F32, tag="pj")
                for c in range(DCH):
                    nc.tensor.matmul(pk, wk_sb[:, c, :], xT_sb[:, c, bass.ts(nt, 512)],
                                     start=(c == 0), stop=(c == DCH - 1))
                nc.vector.tensor_copy(kT2[:, bass.ts(nt, 512)], pk)
            for nt in range(N // 512):
                pq = proj_psum.tile([128, 512], F32, tag="pj")
                for c in range(DCH):
                    nc.tensor.matmul(pq, wq_sb[:, c, :], xT_sb[:, c, bass.ts(nt, 512)],
                                     start=(c == 0), stop=(c == DCH - 1))
                nc.vector.tensor_copy(qT2[:, bass.ts(nt, 512)], pq)
            # v natural: out[n-tile, hd] = xT-chunk.T @ wv-chunk
            for kc in range(N_KC):
                pv = proj_psum.tile([128, 128], F32, tag="pv")
                for c in range(DCH):
                    nc.tensor.matmul(pv, xT_sb[:, c, bass.ts(kc, 128)], wv_sb[:, c, :],
                                     start=(c == 0), stop=(c == DCH - 1))
                # interleave the two heads' 64-col halves into v2 (cols 0-63, 65-128)
                nc.vector.tensor_copy(v2[:, kc, 0:64], pv[:, 0:64])
                nc.vector.tensor_copy(v2[:, kc, 65:129], pv[:, 64:128])
        # ones columns for the softmax-denominator trick
        nc.vector.memset(v2[:, :, 64], 1.0)
        nc.vector.memset(v2[:, :, 129], 1.0)

        # ---- attention (pipelined: sc(kc) | exp(kc) | av(kc-1)) ----
        with (
            tc.tile_pool(name="sc_psum", bufs=2, space="PSUM") as sc_psum,
            tc.tile_pool(name="av_psum", bufs=2, space="PSUM") as av_psum,
            tc.tile_pool(name="attn_sb", bufs=4) as attn_sb,
            tc.tile_pool(name="avs_pool", bufs=2) as avs_pool,
            tc.tile_pool(name="norm_sb", bufs=2) as norm_sb,
        ):
            rc_t = const.tile([65, 2, QQ_W], F32)   # reciprocal lands in row 64
            for qq in range(N_QQ):
                avs = [av_psum.tile([65, QQ_W], F32, tag="av", name=f"av_{qq}_{h}")
                       for h in range(2)]
                prev_at = None
                for kc in range(N_KC):
                    # -- sc MMs for kc (h0/h1 interleaved: alternating row groups) --
                    scs = [sc_psum.tile([128, QQ_W], F32, tag="sc",
                                        name=f"sc_{qq}_{kc}_{h}") for h in range(2)]
                    for s in range(2):
                        for h in range(2):
                            nc.tensor.matmul(
                                scs[h][:, bass.ts(s, 512)],
                                kT2[h * 64:(h + 1) * 64, bass.ts(kc, 128)],
                                qT2[h * 64:(h + 1) * 64,
                                    qq * QQ_W + s * 512:qq * QQ_W + (s + 1) * 512],
                                start=True, stop=True)
                    # -- exps for kc: ACT half / DVE half, parity-swapped --
                    sa = kc % 2          # ACT does s-half sa, DVE the other
                    sd = 1 - sa
                    ats = []
                    for h in range(2):
                        at = attn_sb.tile([128, QQ_W], BF16, tag="at",
                                          name=f"at_{qq}_{kc}_{h}")
                        ats.append(at)
                        nc.scalar.activation(
                            at[:, bass.ts(sa, 512)], scs[h][:, bass.ts(sa, 512)],
                            mybir.ActivationFunctionType.Exp, scale=float(SCALE))
                        if EXP_MODE == "split":
                            nc.vector.tensor_scalar(
                                at[:, bass.ts(sd, 512)].bitcast(I16),
                                scs[h][:, bass.ts(sd, 512)],
                                A_SCH, B_SCH,
                                mybir.AluOpType.mult, mybir.AluOpType.add)
                        else:
                            nc.scalar.activation(
                                at[:, bass.ts(sd, 512)], scs[h][:, bass.ts(sd, 512)],
                                mybir.ActivationFunctionType.Exp, scale=float(SCALE))
                    # -- av MMs for kc-1 --
                    if prev_at is not None:
                        pkc = kc - 1
                        for s in range(2):
                            for h in range(2):
                                nc.tensor.matmul(
                                    avs[h][:, bass.ts(s, 512)],
                                    v2[:, pkc, h * 65:(h + 1) * 65],
                                    prev_at[h][:, bass.ts(s, 512)],
                                    start=(pkc == 0), stop=(pkc == N_KC - 1))
                    prev_at = ats
                # trailing av for kc = N_KC-1
                for s in range(2):
                    for h in range(2):
                        nc.tensor.matmul(
                            avs[h][:, bass.ts(s, 512)],
                            v2[:, N_KC - 1, h * 65:(h + 1) * 65],
                            prev_at[h][:, bass.ts(s, 512)],
                            start=False, stop=True)
                # -- drain + normalize --
                avs_sb = avs_pool.tile([65, 2, QQ_W], F32, tag="avs",
                                       name=f"avs_{qq}")
                for h in range(2):
                    nc.scalar.copy(avs_sb[:, h, :], avs[h])
                for h in range(2):
                    if RECIP_MODE == "approx":
                        nc.vector.reciprocal_approx_fast(rc_t[64:65, h, :],
                                                         avs_sb[64:65, h, :])
                    else:
                        nc.vector.reciprocal(rc_t[64:65, h, :],
                                             avs_sb[64:65, h, :])
                    nc.sync.dma_start(out=recip_dram[qq:qq + 1, h, :],
                                      in_=rc_t[64:65, h, :])
                    bc = norm_sb.tile([64, QQ_W], F32, tag="bc", name=f"bc_{qq}_{h}")
                    src = recip_dram[qq, h, :]
                    bcast = bass.AP(tensor=src.tensor, offset=src.offset,
                                    ap=[[0, 64]] + src.ap)
                    nc.sync.dma_start(out=bc, in_=bcast)
                    nc.vector.tensor_mul(outT[:, h, qq * QQ_W:(qq + 1) * QQ_W],
                                         avs_sb[0:64, h, :], bc)

        # ---- output projection ----
        with (
            tc.tile_pool(name="op_psum", bufs=3, space="PSUM") as op_psum,
            tc.tile_pool(name="op_sb", bufs=3) as op_sb,
        ):
            for nt in range(N // 128):
                po = op_psum.tile([128, D], F32, tag="po")
                nc.tensor.matmul(po, outT[:, 0, bass.ts(nt, 128)], wo_sb[:, 0, :],
                                 start=True, stop=False)
                nc.tensor.matmul(po, outT[:, 1, bass.ts(nt, 128)], wo_sb[:, 1, :],
                                 start=False, stop=True)
                ob = op_sb.tile([128, D], F32, tag="ob")
                nc.vector.tensor_copy(ob, po)
                nc.sync.dma_start(out=out[bass.ts(nt, 128), :], in_=ob)

    nc.compile()
    return nc


_NC_CACHE = None


def build_in_maps(x, Wq, Wk, Wv, Wo):
    bf = ml_dtypes.bfloat16
    x = np.asarray(x, np.float32)
    Wq, Wk, Wv, Wo = (np.asarray(a, np.float32) for a in (Wq, Wk, Wv, Wo))
    in_maps = []
    for c in range(8):
        b = c // 4
        h0 = 2 * (c % 4)
        xT = np.ascontiguousarray(x[b].T.astype(bf))
        wqT = np.ascontiguousarray(Wq[h0 * 64:(h0 + 2) * 64].T.astype(bf))
        wkT = np.ascontiguousarray(Wk[h0 * 64:(h0 + 2) * 64].T.astype(bf))
        wvT = np.ascontiguousarray(Wv[h0 * 64:(h0 + 2) * 64].T.astype(bf))
        woT = np.stack([np.ascontiguousarray(Wo[:, (h0 + h) * 64:(h0 + h + 1) * 64].T.astype(bf))
                        for h in range(2)])
        in_maps.append({"xT": xT, "wqT": wqT, "wkT": wkT, "wvT": wvT, "woT": woT})
    return in_maps


def kernel(x, Wq, Wk, Wv, Wo, bo):
    global _NC_CACHE
    bo = np.asarray(bo, np.float32)
    in_maps = build_in_maps(x, Wq, Wk, Wv, Wo)

    if _NC_CACHE is None:
        _NC_CACHE = build_bass()
    res = run_bass_kernel_spmd(_NC_CACHE, in_maps, list(range(8)))
    partials = [np.asarray(res.results[c]["out"], np.float32) for c in range(8)]

    out = np.empty((B, N, D), np.float32)
    for b in range(B):
        out[b] = partials[4 * b] + partials[4 * b + 1] + partials[4 * b + 2] + partials[4 * b + 3] + bo
    return out


if __name__ == "__main__":
    nc = build_bass()
    print("built ok")


# revision 10
# speedup vs baseline: 8019.4658x; 1.3566x over previous
"""Self-attention (8 heads, d=64, B=2, N=4096, D=512) on 8 TRN2 NeuronCores.

Sharding: batch*heads across cores — core c handles batch b=c//4, heads
(2*(c%4), 2*(c%4)+1). Projection weights are sliced per-core on the host;
x is pre-transposed on the host so the device needs no transposes at all.

v2: software-pipelined attention loop with the softmax exp split across
BOTH the Scalar (ACT) and Vector (DVE) engines:
  - ACT computes exp(sc*SCALE) for one 512-wide half of each score tile
    (hardware spline, exact).
  - DVE computes the other half with a Schraudolph-style bit-trick:
    bf16_bits(e^x) ~= int16(x * 128*log2e*SCALE + 128*(127-0.0573)),
    emitted as one tensor_scalar (mult,add) with an int16-bitcast write
    into the bf16 attn tile (fp32->int16 conversion rounds-to-nearest).
  The halves alternate with kc parity so every query row mixes exact and
  approximated weights (rel err ~9e-3 vs 2e-2 budget).
Pipelined emission per kc: sc MMs (kc) -> exps (kc) -> av MMs (kc-1), so
the PE never idles waiting on the exp and the HAM clock-gate stays warm.

Device dataflow (per core, fully transposed "scoresT" formulation):
  qT2/kT2 [hd=128, n]  = W.T-chunks @ xT-chunks          (PE)
  v2      [n, hd+ones] natural                            (PE, bf16 store)
  per qq (1024 queries), kc (128 keys), h (2 heads):
    scT psum[128k,1024q] = kh.T @ qh   (interleaved h0/h1 -> row-group pairs)
    attnT = exp(scT*SCALE) -> bf16 SBUF   (ACT half | DVE half)
    av[65,1024] += v2'[kc].T @ attnT      (PE, accumulate; ones col = denom)
  drain av -> SBUF (ScalarE copy), reciprocal_approx_fast on denom row,
  DMA-broadcast, normalize mul -> outT (DVE)
  partial[n,512] = sum_h outT[h].T @ woT[h]               (PE)
Host: out[b] = sum of its 4 cores' partials + bo.
"""
import numpy as np
import ml_dtypes
from contextlib import ExitStack

import concourse.bass as bass
from concourse import bacc
import concourse.mybir as mybir
import concourse.tile as tile
from concourse.bass_utils import run_bass_kernel_spmd

B, N, D = 2, 4096, 512
HEADS, DH = 8, 64
SCALE = DH ** -0.5

F32 = mybir.dt.float32
F32R = mybir.dt.bfloat16  # matmul operand dtype (bf16: 1cyc/row)
BF16 = mybir.dt.bfloat16
I16 = mybir.dt.int16

QQ_W = 1024          # q-chunk width in the attention loop
N_QQ = N // QQ_W     # 4
N_KC = N // 128      # 32 key chunks
DCH = D // 128       # 4 contraction chunks for projections

LOG2E = 1.4426950408889634
A_SCH = float(128.0 * LOG2E * SCALE)          # fold attention scale in
B_SCH = float(128.0 * (127.0 - 0.057304959))  # equal-ripple bias

EXP_MODE = "whole"   # 'whole' = h0 tile on ACT, h1 tile on DVE Schraudolph;
                     # 'split' = each tile half ACT / half DVE; 'act' = all ACT
RECIP_MODE = "dma"   # 'dma' = exact reciprocal on a [128,8] DMA-reshaped view;
                     # 'exact' = nc.vector.reciprocal on [1,1024]


def build_bass():
    nc = bacc.Bacc(None, target_bir_lowering=False)

    xT = nc.dram_tensor("xT", [D, N], F32R, kind="ExternalInput")
    wqT = nc.dram_tensor("wqT", [D, 128], F32R, kind="ExternalInput")
    wkT = nc.dram_tensor("wkT", [D, 128], F32R, kind="ExternalInput")
    wvT = nc.dram_tensor("wvT", [D, 128], F32R, kind="ExternalInput")
    woT = nc.dram_tensor("woT", [2, 64, D], F32R, kind="ExternalInput")
    out = nc.dram_tensor("out", [N, D], F32, kind="ExternalOutput")
    recip_dram = nc.dram_tensor("recip_scratch", [N_QQ, 2, QQ_W], F32)
    denom_dram = nc.dram_tensor("denom_scratch", [N_QQ, 2, QQ_W], F32)

    with tile.TileContext(nc) as tc, ExitStack() as ctx:
        const = ctx.enter_context(tc.tile_pool(name="const", bufs=1))

        # ---- load inputs (xT chunked along n so projections start early) ----
        xT_sb = const.tile([128, DCH, N], F32R)            # xT[(c p), n] -> [p, c, n]
        xT_r = xT.rearrange("(c p) n -> p c n", p=128)
        for nch in range(4):
            nc.sync.dma_start(out=xT_sb[:, :, bass.ts(nch, N // 4)],
                              in_=xT_r[:, :, bass.ts(nch, N // 4)])
        wq_sb = const.tile([128, DCH, 128], F32R)
        nc.sync.dma_start(out=wq_sb, in_=wqT.rearrange("(c p) m -> p c m", p=128))
        wk_sb = const.tile([128, DCH, 128], F32R)
        nc.sync.dma_start(out=wk_sb, in_=wkT.rearrange("(c p) m -> p c m", p=128))
        wv_sb = const.tile([128, DCH, 128], F32R)
        nc.sync.dma_start(out=wv_sb, in_=wvT.rearrange("(c p) m -> p c m", p=128))
        wo_sb = const.tile([64, 2, D], F32R)
        nc.sync.dma_start(out=wo_sb, in_=woT.rearrange("h d n -> d h n"))

        qT2 = const.tile([128, N], F32R)                   # [2-head d, n]
        kT2 = const.tile([128, N], F32R)
        v2 = const.tile([128, N_KC, 130], BF16)            # [k-part, kc, (v_h0|1|v_h1|1)]
        outT = const.tile([64, 2, N], F32R)                # normalized per-head av

        # ---- projections ----
        with tc.tile_pool(name="proj_psum", bufs=3, space="PSUM") as proj_psum:
            for nt in range(N // 512):
                pk = proj_psum.tile([128, 512], F32, tag="pj")
                for c in range(DCH):
                    nc.tensor.matmul(pk, wk_sb[:, c, :], xT_sb[:, c, bass.ts(nt, 512)],
                                     start=(c == 0), stop=(c == DCH - 1))
                nc.vector.tensor_copy(kT2[:, bass.ts(nt, 512)], pk)
            for nt in range(N // 512):
                pq = proj_psum.tile([128, 512], F32, tag="pj")
                for c in range(DCH):
                    nc.tensor.matmul(pq, wq_sb[:, c, :], xT_sb[:, c, bass.ts(nt, 512)],
                                     start=(c == 0), stop=(c == DCH - 1))
                nc.vector.tensor_copy(qT2[:, bass.ts(nt, 512)], pq)
            # v natural: out[n-tile, hd] = xT-chunk.T @ wv-chunk
            for kc in range(N_KC):
                pv = proj_psum.tile([128, 128], F32, tag="pv")
                for c in range(DCH):
                    nc.tensor.matmul(pv, xT_sb[:, c, bass.ts(kc, 128)], wv_sb[:, c, :],
                                     start=(c == 0), stop=(c == DCH - 1))
                # interleave the two heads' 64-col halves into v2 (cols 0-63, 65-128)
                nc.vector.tensor_copy(v2[:, kc, 0:64], pv[:, 0:64])
                nc.vector.tensor_copy(v2[:, kc, 65:129], pv[:, 64:128])
        # ones columns for the softmax-denominator trick
        nc.vector.memset(v2[:, :, 64], 1.0)
        nc.vector.memset(v2[:, :, 129], 1.0)

        # ---- attention (pipelined: sc(kc) | exp(kc) | av(kc-1)) ----
        with (
            tc.tile_pool(name="sc_psum", bufs=2, space="PSUM") as sc_psum,
            tc.tile_pool(name="av_psum", bufs=2, space="PSUM") as av_psum,
            tc.tile_pool(name="attn_sb", bufs=4) as attn_sb,
            tc.tile_pool(name="avs_pool", bufs=2) as avs_pool,
            tc.tile_pool(name="norm_sb", bufs=2) as norm_sb,
        ):
            rc_t = const.tile([65, 2, QQ_W], F32)   # reciprocal lands in row 64
            for qq in range(N_QQ):
                avs = [av_psum.tile([65, QQ_W], F32, tag="av", name=f"av_{qq}_{h}")
                       for h in range(2)]
                prev_at = None
                for kc in range(N_KC):
                    # -- sc MMs for kc (h0/h1 interleaved: alternating row groups) --
                    scs = [sc_psum.tile([128, QQ_W], F32, tag="sc",
                                        name=f"sc_{qq}_{kc}_{h}") for h in range(2)]
                    for s in range(2):
                        for h in range(2):
                            nc.tensor.matmul(
                                scs[h][:, bass.ts(s, 512)],
                                kT2[h * 64:(h + 1) * 64, bass.ts(kc, 128)],
                                qT2[h * 64:(h + 1) * 64,
                                    qq * QQ_W + s * 512:qq * QQ_W + (s + 1) * 512],
                                start=True, stop=True)
                    # -- exps for kc --
                    ats = []
                    for h in range(2):
                        at = attn_sb.tile([128, QQ_W], BF16, tag="at",
                                          name=f"at_{qq}_{kc}_{h}")
                        ats.append(at)
                        if EXP_MODE == "whole":
                            # h0 whole tile on ACT; h1 whole tile on DVE
                            if h == 0:
                                nc.scalar.activation(
                                    at, scs[h],
                                    mybir.ActivationFunctionType.Exp,
                                    scale=float(SCALE))
                            else:
                                nc.vector.tensor_scalar(
                                    at.bitcast(I16), scs[h],
                                    A_SCH, B_SCH,
                                    mybir.AluOpType.mult, mybir.AluOpType.add)
                            continue
                        sa = kc % 2          # ACT does s-half sa, DVE the other
                        sd = 1 - sa
                        nc.scalar.activation(
                            at[:, bass.ts(sa, 512)], scs[h][:, bass.ts(sa, 512)],
                            mybir.ActivationFunctionType.Exp, scale=float(SCALE))
                        if EXP_MODE == "split":
                            nc.vector.tensor_scalar(
                                at[:, bass.ts(sd, 512)].bitcast(I16),
                                scs[h][:, bass.ts(sd, 512)],
                                A_SCH, B_SCH,
                                mybir.AluOpType.mult, mybir.AluOpType.add)
                        else:
                            nc.scalar.activation(
                                at[:, bass.ts(sd, 512)], scs[h][:, bass.ts(sd, 512)],
                                mybir.ActivationFunctionType.Exp, scale=float(SCALE))
                    # -- av MMs for kc-1 --
                    if prev_at is not None:
                        pkc = kc - 1
                        for s in range(2):
                            for h in range(2):
                                nc.tensor.matmul(
                                    avs[h][:, bass.ts(s, 512)],
                                    v2[:, pkc, h * 65:(h + 1) * 65],
                                    prev_at[h][:, bass.ts(s, 512)],
                                    start=(pkc == 0), stop=(pkc == N_KC - 1))
                    prev_at = ats
                # trailing av for kc = N_KC-1
                for s in range(2):
                    for h in range(2):
                        nc.tensor.matmul(
                            avs[h][:, bass.ts(s, 512)],
                            v2[:, N_KC - 1, h * 65:(h + 1) * 65],
                            prev_at[h][:, bass.ts(s, 512)],
                            start=False, stop=True)
                # -- drain + normalize --
                avs_sb = avs_pool.tile([65, 2, QQ_W], F32, tag="avs",
                                       name=f"avs_{qq}")
                for h in range(2):
                    nc.scalar.copy(avs_sb[:, h, :], avs[h])
                for h in range(2):
                    if RECIP_MODE == "dma":
                        # exact reciprocal across 128 lanes: round-trip the
                        # denominator row through DRAM as a [128, 8] view
                        nc.sync.dma_start(out=denom_dram[qq:qq + 1, h, :],
                                          in_=avs_sb[64:65, h, :])
                        dsrc = denom_dram[qq, h, :]
                        d2d = bass.AP(tensor=dsrc.tensor, offset=dsrc.offset,
                                      ap=[[8, 128], [1, 8]])
                        rin = norm_sb.tile([128, 8], F32, tag="rin",
                                           name=f"rin_{qq}_{h}")
                        nc.sync.dma_start(out=rin, in_=d2d)
                        rout = norm_sb.tile([128, 8], F32, tag="rout",
                                            name=f"rout_{qq}_{h}")
                        nc.vector.reciprocal(rout, rin)
                        rdst = recip_dram[qq, h, :]
                        r2d = bass.AP(tensor=rdst.tensor, offset=rdst.offset,
                                      ap=[[8, 128], [1, 8]])
                        nc.sync.dma_start(out=r2d, in_=rout)
                    else:
                        nc.vector.reciprocal(rc_t[64:65, h, :],
                                             avs_sb[64:65, h, :])
                        nc.sync.dma_start(out=recip_dram[qq:qq + 1, h, :],
                                          in_=rc_t[64:65, h, :])
                    bc = norm_sb.tile([64, QQ_W], F32, tag="bc", name=f"bc_{qq}_{h}")
                    src = recip_dram[qq, h, :]
                    bcast = bass.AP(tensor=src.tensor, offset=src.offset,
                                    ap=[[0, 64]] + src.ap)
                    nc.sync.dma_start(out=bc, in_=bcast)
                    nc.vector.tensor_mul(outT[:, h, qq * QQ_W:(qq + 1) * QQ_W],
                                         avs_sb[0:64, h, :], bc)

        # ---- output projection ----
        with (
            tc.tile_pool(name="op_psum", bufs=3, space="PSUM") as op_psum,
            tc.tile_pool(name="op_sb", bufs=3) as op_sb,
        ):
            for nt in range(N // 128):
                po = op_psum.tile([128, D], F32, tag="po")
                nc.tensor.matmul(po, outT[:, 0, bass.ts(nt, 128)], wo_sb[:, 0, :],
                                 start=True, stop=False)
                nc.tensor.matmul(po, outT[:, 1, bass.ts(nt, 128)], wo_sb[:, 1, :],
                                 start=False, stop=True)
                ob = op_sb.tile([128, D], F32, tag="ob")
                nc.vector.tensor_copy(ob, po)
                nc.sync.dma_start(out=out[bass.ts(nt, 128), :], in_=ob)

    nc.compile()
    return nc


_NC_CACHE = None


def build_in_maps(x, Wq, Wk, Wv, Wo):
    bf = ml_dtypes.bfloat16
    x = np.asarray(x, np.float32)
    Wq, Wk, Wv, Wo = (np.asarray(a, np.float32) for a in (Wq, Wk, Wv, Wo))
    in_maps = []
    for c in range(8):
        b = c // 4
        h0 = 2 * (c % 4)
        xT = np.ascontiguousarray(x[b].T.astype(bf))
        wqT = np.ascontiguousarray(Wq[h0 * 64:(h0 + 2) * 64].T.astype(bf))
        wkT = np.ascontiguousarray(Wk[h0 * 64:(h0 + 2) * 64].T.astype(bf))
        wvT = np.ascontiguousarray(Wv[h0 * 64:(h0 + 2) * 64].T.astype(bf))
        woT = np.stack([np.ascontiguousarray(Wo[:, (h0 + h) * 64:(h0 + h + 1) * 64].T.astype(bf))
                        for h in range(2)])
        in_maps.append({"xT": xT, "wqT": wqT, "wkT": wkT, "wvT": wvT, "woT": woT})
    return in_maps


def kernel(x, Wq, Wk, Wv, Wo, bo):
    global _NC_CACHE
    bo = np.asarray(bo, np.float32)
    in_maps = build_in_maps(x, Wq, Wk, Wv, Wo)

    if _NC_CACHE is None:
        _NC_CACHE = build_bass()
    res = run_bass_kernel_spmd(_NC_CACHE, in_maps, list(range(8)))
    partials = [np.asarray(res.results[c]["out"], np.float32) for c in range(8)]

    out = np.empty((B, N, D), np.float32)
    for b in range(B):
        out[b] = partials[4 * b] + partials[4 * b + 1] + partials[4 * b + 2] + partials[4 * b + 3] + bo
    return out


if __name__ == "__main__":
    nc = build_bass()
    print("built ok")


# revision 12
# speedup vs baseline: 9319.0660x; 1.1621x over previous
"""Self-attention (8 heads, d=64, B=2, N=4096, D=512) on 8 TRN2 NeuronCores.

Sharding: batch*heads across cores — core c handles batch b=c//4, heads
(2*(c%4), 2*(c%4)+1). Projection weights are sliced per-core on the host;
x is pre-transposed on the host so the device needs no transposes at all.

v2: software-pipelined attention loop with the softmax exp split across
BOTH the Scalar (ACT) and Vector (DVE) engines:
  - ACT computes exp(sc*SCALE) for one 512-wide half of each score tile
    (hardware spline, exact).
  - DVE computes the other half with a Schraudolph-style bit-trick:
    bf16_bits(e^x) ~= int16(x * 128*log2e*SCALE + 128*(127-0.0573)),
    emitted as one tensor_scalar (mult,add) with an int16-bitcast write
    into the bf16 attn tile (fp32->int16 conversion rounds-to-nearest).
  The halves alternate with kc parity so every query row mixes exact and
  approximated weights (rel err ~9e-3 vs 2e-2 budget).
Pipelined emission per kc: sc MMs (kc) -> exps (kc) -> av MMs (kc-1), so
the PE never idles waiting on the exp and the HAM clock-gate stays warm.

Device dataflow (per core, fully transposed "scoresT" formulation):
  qT2/kT2 [hd=128, n]  = W.T-chunks @ xT-chunks          (PE)
  v2      [n, hd+ones] natural                            (PE, bf16 store)
  per qq (1024 queries), kc (128 keys), h (2 heads):
    scT psum[128k,1024q] = kh.T @ qh   (interleaved h0/h1 -> row-group pairs)
    attnT = exp(scT*SCALE) -> bf16 SBUF   (ACT half | DVE half)
    av[65,1024] += v2'[kc].T @ attnT      (PE, accumulate; ones col = denom)
  drain av -> SBUF (ScalarE copy), reciprocal_approx_fast on denom row,
  DMA-broadcast, normalize mul -> outT (DVE)
  partial[n,512] = sum_h outT[h].T @ woT[h]               (PE)
Host: out[b] = sum of its 4 cores' partials + bo.
"""
import numpy as np
import ml_dtypes
from contextlib import ExitStack

import concourse.bass as bass
from concourse import bacc
import concourse.mybir as mybir
import concourse.tile as tile
from concourse.bass_utils import run_bass_kernel_spmd

B, N, D = 2, 4096, 512
HEADS, DH = 8, 64
SCALE = DH ** -0.5

F32 = mybir.dt.float32
F32R = mybir.dt.bfloat16  # matmul operand dtype (bf16: 1cyc/row)
BF16 = mybir.dt.bfloat16
I16 = mybir.dt.int16

QQ_W = 1024          # q-chunk width in the attention loop
N_QQ = N // QQ_W     # 4
N_KC = N // 128      # 32 key chunks
DCH = D // 128       # 4 contraction chunks for projections

LOG2E = 1.4426950408889634
A_SCH = float(128.0 * LOG2E * SCALE)          # fold attention scale in
B_SCH = float(128.0 * (127.0 - 0.057304959))  # equal-ripple bias

EXP_MODE = "whole"   # 'whole' = h0 tile on ACT, h1 tile on DVE Schraudolph;
                     # 'split' = each tile half ACT / half DVE; 'act' = all ACT
RECIP_MODE = "dma"   # 'dma' = exact reciprocal on a [128,8] DMA-reshaped view;
                     # 'exact' = nc.vector.reciprocal on [1,1024]


def build_bass():
    nc = bacc.Bacc(None, target_bir_lowering=False)

    xT = nc.dram_tensor("xT", [D, N], F32R, kind="ExternalInput")
    wqT = nc.dram_tensor("wqT", [D, 128], F32R, kind="ExternalInput")
    wkT = nc.dram_tensor("wkT", [D, 128], F32R, kind="ExternalInput")
    wvT = nc.dram_tensor("wvT", [D, 128], F32R, kind="ExternalInput")
    woT = nc.dram_tensor("woT", [2, 64, D], F32R, kind="ExternalInput")
    out = nc.dram_tensor("out", [N, D], F32, kind="ExternalOutput")
    recip_dram = nc.dram_tensor("recip_scratch", [N_QQ, 2, QQ_W], F32)
    denom_dram = nc.dram_tensor("denom_scratch", [N_QQ, 2, QQ_W], F32)

    with tile.TileContext(nc) as tc, ExitStack() as ctx:
        const = ctx.enter_context(tc.tile_pool(name="const", bufs=1))

        # ---- load inputs (xT chunked along n so projections start early) ----
        xT_sb = const.tile([128, DCH, N], F32R)            # xT[(c p), n] -> [p, c, n]
        xT_r = xT.rearrange("(c p) n -> p c n", p=128)
        for nch in range(4):
            nc.sync.dma_start(out=xT_sb[:, :, bass.ts(nch, N // 4)],
                              in_=xT_r[:, :, bass.ts(nch, N // 4)])
        wq_sb = const.tile([128, DCH, 128], F32R)
        nc.sync.dma_start(out=wq_sb, in_=wqT.rearrange("(c p) m -> p c m", p=128))
        wk_sb = const.tile([128, DCH, 128], F32R)
        nc.sync.dma_start(out=wk_sb, in_=wkT.rearrange("(c p) m -> p c m", p=128))
        wv_sb = const.tile([128, DCH, 128], F32R)
        nc.sync.dma_start(out=wv_sb, in_=wvT.rearrange("(c p) m -> p c m", p=128))
        wo_sb = const.tile([64, 2, D], F32R)
        nc.sync.dma_start(out=wo_sb, in_=woT.rearrange("h d n -> d h n"))

        qT2 = const.tile([128, N], F32R)                   # [2-head d, n]
        kT2 = const.tile([128, N], F32R)
        v2 = const.tile([128, N_KC, 130], BF16)            # [k-part, kc, (v_h0|1|v_h1|1)]
        outT = const.tile([64, 2, N], F32R)                # normalized per-head av

        # ---- projections ----
        with tc.tile_pool(name="proj_psum", bufs=3, space="PSUM") as proj_psum:
            for nt in range(N // 512):
                pk = proj_psum.tile([128, 512], F32, tag="pj")
                for c in range(DCH):
                    nc.tensor.matmul(pk, wk_sb[:, c, :], xT_sb[:, c, bass.ts(nt, 512)],
                                     start=(c == 0), stop=(c == DCH - 1))
                nc.vector.tensor_copy(kT2[:, bass.ts(nt, 512)], pk)
            for nt in range(N // 512):
                pq = proj_psum.tile([128, 512], F32, tag="pj")
                for c in range(DCH):
                    nc.tensor.matmul(pq, wq_sb[:, c, :], xT_sb[:, c, bass.ts(nt, 512)],
                                     start=(c == 0), stop=(c == DCH - 1))
                nc.vector.tensor_copy(qT2[:, bass.ts(nt, 512)], pq)
            # v natural: out[n-tile, hd] = xT-chunk.T @ wv-chunk
            for kc in range(N_KC):
                pv = proj_psum.tile([128, 128], F32, tag="pv")
                for c in range(DCH):
                    nc.tensor.matmul(pv, xT_sb[:, c, bass.ts(kc, 128)], wv_sb[:, c, :],
                                     start=(c == 0), stop=(c == DCH - 1))
                # interleave the two heads' 64-col halves into v2 (cols 0-63, 65-128)
                nc.vector.tensor_copy(v2[:, kc, 0:64], pv[:, 0:64])
                nc.vector.tensor_copy(v2[:, kc, 65:129], pv[:, 64:128])
        # ones columns for the softmax-denominator trick
        nc.vector.memset(v2[:, :, 64], 1.0)
        nc.vector.memset(v2[:, :, 129], 1.0)

        # ---- attention (pipelined: sc(kc) | exp(kc) | av(kc-1)) ----
        with (
            tc.tile_pool(name="sc_psum", bufs=2, space="PSUM") as sc_psum,
            tc.tile_pool(name="av_psum", bufs=2, space="PSUM") as av_psum,
            tc.tile_pool(name="attn_sb", bufs=4) as attn_sb,
            tc.tile_pool(name="avs_pool", bufs=2) as avs_pool,
            tc.tile_pool(name="norm_sb", bufs=2) as norm_sb,
        ):
            # (EXP_MODE/RECIP_MODE now fixed: whole-tile exps, dma-reshape recip)
            def emit_sc(qq, kc, scs):
                for s in range(2):
                    for h in range(2):
                        nc.tensor.matmul(
                            scs[h][:, bass.ts(s, 512)],
                            kT2[h * 64:(h + 1) * 64, bass.ts(kc, 128)],
                            qT2[h * 64:(h + 1) * 64,
                                qq * QQ_W + s * 512:qq * QQ_W + (s + 1) * 512],
                            start=True, stop=True)

            def emit_exps(qq, kc, scs):
                ats = []
                for h in range(2):
                    at = attn_sb.tile([128, QQ_W], BF16, tag="at",
                                      name=f"at_{qq}_{kc}_{h}")
                    ats.append(at)
                    if h == 0:
                        nc.scalar.activation(
                            at, scs[h], mybir.ActivationFunctionType.Exp,
                            scale=float(SCALE))
                    else:
                        nc.vector.tensor_scalar(
                            at.bitcast(I16), scs[h], A_SCH, B_SCH,
                            mybir.AluOpType.mult, mybir.AluOpType.add)
                return ats

            def emit_av(avs, pkc, p_at):
                for s in range(2):
                    for h in range(2):
                        nc.tensor.matmul(
                            avs[h][:, bass.ts(s, 512)],
                            v2[:, pkc, h * 65:(h + 1) * 65],
                            p_at[h][:, bass.ts(s, 512)],
                            start=(pkc == 0), stop=(pkc == N_KC - 1))

            # deferred per-qq normalize work, spread one step per kc so the
            # PE never starves at qq boundaries (HAM stays warm)
            def norm_steps(qq, avs):
                avs_sb = avs_pool.tile([65, 2, QQ_W], F32, tag="avs",
                                       name=f"avs_{qq}")
                # step 0: drain h0 on ACT, h1 on DVE (frees av psum banks)
                yield lambda: (nc.scalar.copy(avs_sb[:, 0, :], avs[0]),
                               nc.vector.tensor_copy(avs_sb[:, 1, :], avs[1]))

                def recip(h):
                    nc.sync.dma_start(out=denom_dram[qq:qq + 1, h, :],
                                      in_=avs_sb[64:65, h, :])
                    dsrc = denom_dram[qq, h, :]
                    d2d = bass.AP(tensor=dsrc.tensor, offset=dsrc.offset,
                                  ap=[[8, 128], [1, 8]])
                    rin = norm_sb.tile([128, 8], F32, tag="rin",
                                       name=f"rin_{qq}_{h}")
                    nc.sync.dma_start(out=rin, in_=d2d)
                    rout = norm_sb.tile([128, 8], F32, tag="rout",
                                        name=f"rout_{qq}_{h}")
                    nc.vector.reciprocal(rout, rin)
                    rdst = recip_dram[qq, h, :]
                    r2d = bass.AP(tensor=rdst.tensor, offset=rdst.offset,
                                  ap=[[8, 128], [1, 8]])
                    nc.sync.dma_start(out=r2d, in_=rout)
                yield lambda: recip(0)
                yield lambda: recip(1)

                def bcast_mul(h):
                    bc = norm_sb.tile([64, QQ_W], F32, tag="bc",
                                      name=f"bc_{qq}_{h}")
                    src = recip_dram[qq, h, :]
                    bcast = bass.AP(tensor=src.tensor, offset=src.offset,
                                    ap=[[0, 64]] + src.ap)
                    nc.sync.dma_start(out=bc, in_=bcast)
                    nc.vector.tensor_mul(outT[:, h, qq * QQ_W:(qq + 1) * QQ_W],
                                         avs_sb[0:64, h, :], bc)
                yield lambda: bcast_mul(0)
                yield lambda: bcast_mul(1)

            prev_at = None      # at tiles of the previous (qq, kc)
            prev_avs = None     # av accumulators of the previous kc's qq
            pending = []        # deferred normalize steps from the last qq
            for qq in range(N_QQ):
                avs = [av_psum.tile([65, QQ_W], F32, tag="av", name=f"av_{qq}_{h}")
                       for h in range(2)]
                for kc in range(N_KC):
                    scs = [sc_psum.tile([128, QQ_W], F32, tag="sc",
                                        name=f"sc_{qq}_{kc}_{h}") for h in range(2)]
                    emit_sc(qq, kc, scs)
                    ats = emit_exps(qq, kc, scs)
                    if prev_at is not None:
                        emit_av(prev_avs, (kc - 1) % N_KC, prev_at)
                    if pending:
                        pending.pop(0)()
                    prev_at, prev_avs = ats, avs
                pending = list(norm_steps(qq, avs))
            # tail: trailing av of the last kc, then the last qq's normalize
            emit_av(prev_avs, N_KC - 1, prev_at)
            for step in pending:
                step()

        # ---- output projection ----
        with (
            tc.tile_pool(name="op_psum", bufs=3, space="PSUM") as op_psum,
            tc.tile_pool(name="op_sb", bufs=3) as op_sb,
        ):
            for nt in range(N // 128):
                po = op_psum.tile([128, D], F32, tag="po")
                nc.tensor.matmul(po, outT[:, 0, bass.ts(nt, 128)], wo_sb[:, 0, :],
                                 start=True, stop=False)
                nc.tensor.matmul(po, outT[:, 1, bass.ts(nt, 128)], wo_sb[:, 1, :],
                                 start=False, stop=True)
                ob = op_sb.tile([128, D], F32, tag="ob")
                nc.vector.tensor_copy(ob, po)
                nc.sync.dma_start(out=out[bass.ts(nt, 128), :], in_=ob)

    nc.compile()
    return nc


_NC_CACHE = None


def build_in_maps(x, Wq, Wk, Wv, Wo):
    bf = ml_dtypes.bfloat16
    x = np.asarray(x, np.float32)
    Wq, Wk, Wv, Wo = (np.asarray(a, np.float32) for a in (Wq, Wk, Wv, Wo))
    in_maps = []
    for c in range(8):
        b = c // 4
        h0 = 2 * (c % 4)
        xT = np.ascontiguousarray(x[b].T.astype(bf))
        wqT = np.ascontiguousarray(Wq[h0 * 64:(h0 + 2) * 64].T.astype(bf))
        wkT = np.ascontiguousarray(Wk[h0 * 64:(h0 + 2) * 64].T.astype(bf))
        wvT = np.ascontiguousarray(Wv[h0 * 64:(h0 + 2) * 64].T.astype(bf))
        woT = np.stack([np.ascontiguousarray(Wo[:, (h0 + h) * 64:(h0 + h + 1) * 64].T.astype(bf))
                        for h in range(2)])
        in_maps.append({"xT": xT, "wqT": wqT, "wkT": wkT, "wvT": wvT, "woT": woT})
    return in_maps


def kernel(x, Wq, Wk, Wv, Wo, bo):
    global _NC_CACHE
    bo = np.asarray(bo, np.float32)
    in_maps = build_in_maps(x, Wq, Wk, Wv, Wo)

    if _NC_CACHE is None:
        _NC_CACHE = build_bass()
    res = run_bass_kernel_spmd(_NC_CACHE, in_maps, list(range(8)))
    partials = [np.asarray(res.results[c]["out"], np.float32) for c in range(8)]

    out = np.empty((B, N, D), np.float32)
    for b in range(B):
        out[b] = partials[4 * b] + partials[4 * b + 1] + partials[4 * b + 2] + partials[4 * b + 3] + bo
    return out


if __name__ == "__main__":
    nc = build_bass()
    print("built ok")


# revision 16
# speedup vs baseline: 9495.3366x; 1.0189x over previous
"""Self-attention (8 heads, d=64, B=2, N=4096, D=512) on 8 TRN2 NeuronCores.

Sharding: batch*heads across cores — core c handles batch b=c//4, heads
(2*(c%4), 2*(c%4)+1). Projection weights are sliced per-core on the host;
x is pre-transposed on the host so the device needs no transposes at all.

v2: software-pipelined attention loop with the softmax exp split across
BOTH the Scalar (ACT) and Vector (DVE) engines:
  - ACT computes exp(sc*SCALE) for one 512-wide half of each score tile
    (hardware spline, exact).
  - DVE computes the other half with a Schraudolph-style bit-trick:
    bf16_bits(e^x) ~= int16(x * 128*log2e*SCALE + 128*(127-0.0573)),
    emitted as one tensor_scalar (mult,add) with an int16-bitcast write
    into the bf16 attn tile (fp32->int16 conversion rounds-to-nearest).
  The halves alternate with kc parity so every query row mixes exact and
  approximated weights (rel err ~9e-3 vs 2e-2 budget).
Pipelined emission per kc: sc MMs (kc) -> exps (kc) -> av MMs (kc-1), so
the PE never idles waiting on the exp and the HAM clock-gate stays warm.

Device dataflow (per core, fully transposed "scoresT" formulation):
  qT2/kT2 [hd=128, n]  = W.T-chunks @ xT-chunks          (PE)
  v2      [n, hd+ones] natural                            (PE, bf16 store)
  per qq (1024 queries), kc (128 keys), h (2 heads):
    scT psum[128k,1024q] = kh.T @ qh   (interleaved h0/h1 -> row-group pairs)
    attnT = exp(scT*SCALE) -> bf16 SBUF   (ACT half | DVE half)
    av[65,1024] += v2'[kc].T @ attnT      (PE, accumulate; ones col = denom)
  drain av -> SBUF (ScalarE copy), reciprocal_approx_fast on denom row,
  DMA-broadcast, normalize mul -> outT (DVE)
  partial[n,512] = sum_h outT[h].T @ woT[h]               (PE)
Host: out[b] = sum of its 4 cores' partials + bo.
"""
import numpy as np
import ml_dtypes
from contextlib import ExitStack

import concourse.bass as bass
from concourse import bacc
import concourse.mybir as mybir
import concourse.tile as tile
from concourse.bass_utils import run_bass_kernel_spmd

B, N, D = 2, 4096, 512
HEADS, DH = 8, 64
SCALE = DH ** -0.5

F32 = mybir.dt.float32
F32R = mybir.dt.bfloat16  # matmul operand dtype (bf16: 1cyc/row)
BF16 = mybir.dt.bfloat16
I16 = mybir.dt.int16

QQ_W = 1024          # q-chunk width in the attention loop
N_QQ = N // QQ_W     # 4
N_KC = N // 128      # 32 key chunks
DCH = D // 128       # 4 contraction chunks for projections

LOG2E = 1.4426950408889634
A_SCH = float(128.0 * LOG2E * SCALE)          # fold attention scale in
B_SCH = float(128.0 * (127.0 - 0.057304959))  # equal-ripple bias

EXP_MODE = "whole"   # 'whole' = h0 tile on ACT, h1 tile on DVE Schraudolph;
                     # 'split' = each tile half ACT / half DVE; 'act' = all ACT
RECIP_MODE = "dma"   # 'dma' = exact reciprocal on a [128,8] DMA-reshaped view;
                     # 'exact' = nc.vector.reciprocal on [1,1024]


def build_bass():
    nc = bacc.Bacc(None, target_bir_lowering=False)

    xT = nc.dram_tensor("xT", [D, N], F32R, kind="ExternalInput")
    wqT = nc.dram_tensor("wqT", [D, 128], F32R, kind="ExternalInput")
    wkT = nc.dram_tensor("wkT", [D, 128], F32R, kind="ExternalInput")
    wvT = nc.dram_tensor("wvT", [D, 128], F32R, kind="ExternalInput")
    woT = nc.dram_tensor("woT", [2, 64, D], F32R, kind="ExternalInput")
    out = nc.dram_tensor("out", [N, D], F32, kind="ExternalOutput")
    recip_dram = nc.dram_tensor("recip_scratch", [N_QQ, 2, QQ_W], F32)
    denom_dram = nc.dram_tensor("denom_scratch", [N_QQ, 2, QQ_W], F32)

    with tile.TileContext(nc) as tc, ExitStack() as ctx:
        const = ctx.enter_context(tc.tile_pool(name="const", bufs=1))

        # ---- load inputs: tiny weights FIRST so the first proj matmul only
        # waits for them + the first xT chunk, not the whole 4MB of x ----
        wk_sb = const.tile([128, DCH, 128], F32R)
        nc.sync.dma_start(out=wk_sb, in_=wkT.rearrange("(c p) m -> p c m", p=128))
        wq_sb = const.tile([128, DCH, 128], F32R)
        nc.sync.dma_start(out=wq_sb, in_=wqT.rearrange("(c p) m -> p c m", p=128))
        wv_sb = const.tile([128, DCH, 128], F32R)
        nc.sync.dma_start(out=wv_sb, in_=wvT.rearrange("(c p) m -> p c m", p=128))
        wo_sb = const.tile([64, 2, D], F32R)
        nc.sync.dma_start(out=wo_sb, in_=woT.rearrange("h d n -> d h n"))
        xT_sb = const.tile([128, DCH, N], F32R)            # xT[(c p), n] -> [p, c, n]
        xT_r = xT.rearrange("(c p) n -> p c n", p=128)
        for nch in range(8):
            nc.sync.dma_start(out=xT_sb[:, :, bass.ts(nch, N // 8)],
                              in_=xT_r[:, :, bass.ts(nch, N // 8)])

        qT2 = const.tile([128, N], F32R)                   # [2-head d, n]
        kT2 = const.tile([128, N], F32R)
        v2 = const.tile([128, N_KC, 130], BF16)            # [k-part, kc, (v_h0|1|v_h1|1)]
        outT = const.tile([64, 2, N], F32R)                # normalized per-head av

        # ---- projections ----
        with tc.tile_pool(name="proj_psum", bufs=3, space="PSUM") as proj_psum:
            for nt in range(N // 512):
                pk = proj_psum.tile([128, 512], F32, tag="pj")
                for c in range(DCH):
                    nc.tensor.matmul(pk, wk_sb[:, c, :], xT_sb[:, c, bass.ts(nt, 512)],
                                     start=(c == 0), stop=(c == DCH - 1))
                nc.vector.tensor_copy(kT2[:, bass.ts(nt, 512)], pk)
            for nt in range(N // 512):
                pq = proj_psum.tile([128, 512], F32, tag="pj")
                for c in range(DCH):
                    nc.tensor.matmul(pq, wq_sb[:, c, :], xT_sb[:, c, bass.ts(nt, 512)],
                                     start=(c == 0), stop=(c == DCH - 1))
                nc.vector.tensor_copy(qT2[:, bass.ts(nt, 512)], pq)
            # v natural: out[n-tile, hd] = xT-chunk.T @ wv-chunk
            for kc in range(N_KC):
                pv = proj_psum.tile([128, 128], F32, tag="pv")
                for c in range(DCH):
                    nc.tensor.matmul(pv, xT_sb[:, c, bass.ts(kc, 128)], wv_sb[:, c, :],
                                     start=(c == 0), stop=(c == DCH - 1))
                # interleave the two heads' 64-col halves into v2 (cols 0-63, 65-128)
                nc.vector.tensor_copy(v2[:, kc, 0:64], pv[:, 0:64])
                nc.vector.tensor_copy(v2[:, kc, 65:129], pv[:, 64:128])
        # ones columns for the softmax-denominator trick
        nc.vector.memset(v2[:, :, 64], 1.0)
        nc.vector.memset(v2[:, :, 129], 1.0)

        # ---- attention (pipelined: sc(kc) | exp(kc) | av(kc-1)) ----
        with (
            tc.tile_pool(name="sc_psum", bufs=2, space="PSUM") as sc_psum,
            tc.tile_pool(name="av_psum", bufs=2, space="PSUM") as av_psum,
            tc.tile_pool(name="attn_sb", bufs=4) as attn_sb,
            tc.tile_pool(name="avs_pool", bufs=2) as avs_pool,
            tc.tile_pool(name="norm_sb", bufs=2) as norm_sb,
        ):
            # (EXP_MODE/RECIP_MODE now fixed: whole-tile exps, dma-reshape recip)
            # h-grouped emission: consecutive same-weight MMs skip the ~90ns
            # exposed LDWEIGHTS cost of a weight change
            def emit_sc(qq, kc, scs):
                for h in range(2):
                    for s in range(2):
                        nc.tensor.matmul(
                            scs[h][:, bass.ts(s, 512)],
                            kT2[h * 64:(h + 1) * 64, bass.ts(kc, 128)],
                            qT2[h * 64:(h + 1) * 64,
                                qq * QQ_W + s * 512:qq * QQ_W + (s + 1) * 512],
                            start=True, stop=True)

            def emit_exps(qq, kc, scs):
                ats = []
                for h in range(2):
                    at = attn_sb.tile([128, QQ_W], BF16, tag="at",
                                      name=f"at_{qq}_{kc}_{h}")
                    ats.append(at)
                    if h == 0:
                        nc.scalar.activation(
                            at, scs[h], mybir.ActivationFunctionType.Exp,
                            scale=float(SCALE))
                    else:
                        nc.vector.tensor_scalar(
                            at.bitcast(I16), scs[h], A_SCH, B_SCH,
                            mybir.AluOpType.mult, mybir.AluOpType.add)
                return ats

            def emit_av(avs, pkc, p_at):
                for h in range(2):
                    for s in range(2):
                        nc.tensor.matmul(
                            avs[h][:, bass.ts(s, 512)],
                            v2[:, pkc, h * 65:(h + 1) * 65],
                            p_at[h][:, bass.ts(s, 512)],
                            start=(pkc == 0), stop=(pkc == N_KC - 1))

            # deferred per-qq normalize work, spread one step per kc so the
            # PE never starves at qq boundaries (HAM stays warm)
            def norm_steps(qq, avs):
                avs_sb = avs_pool.tile([65, 2, QQ_W], F32, tag="avs",
                                       name=f"avs_{qq}")
                # step 0: drain h0 on ACT, h1 on DVE (frees av psum banks)
                yield lambda: (nc.scalar.copy(avs_sb[:, 0, :], avs[0]),
                               nc.vector.tensor_copy(avs_sb[:, 1, :], avs[1]))

                def recip(h):
                    nc.sync.dma_start(out=denom_dram[qq:qq + 1, h, :],
                                      in_=avs_sb[64:65, h, :])
                    dsrc = denom_dram[qq, h, :]
                    d2d = bass.AP(tensor=dsrc.tensor, offset=dsrc.offset,
                                  ap=[[8, 128], [1, 8]])
                    rin = norm_sb.tile([128, 8], F32, tag="rin",
                                       name=f"rin_{qq}_{h}")
                    nc.sync.dma_start(out=rin, in_=d2d)
                    rout = norm_sb.tile([128, 8], F32, tag="rout",
                                        name=f"rout_{qq}_{h}")
                    nc.vector.reciprocal(rout, rin)
                    rdst = recip_dram[qq, h, :]
                    r2d = bass.AP(tensor=rdst.tensor, offset=rdst.offset,
                                  ap=[[8, 128], [1, 8]])
                    nc.sync.dma_start(out=r2d, in_=rout)
                yield lambda: recip(0)
                yield lambda: recip(1)

                def bcast_mul(h):
                    bc = norm_sb.tile([64, QQ_W], F32, tag="bc",
                                      name=f"bc_{qq}_{h}")
                    src = recip_dram[qq, h, :]
                    bcast = bass.AP(tensor=src.tensor, offset=src.offset,
                                    ap=[[0, 64]] + src.ap)
                    nc.sync.dma_start(out=bc, in_=bcast)
                    nc.vector.tensor_mul(outT[:, h, qq * QQ_W:(qq + 1) * QQ_W],
                                         avs_sb[0:64, h, :], bc)
                yield lambda: bcast_mul(0)
                yield lambda: bcast_mul(1)

            prev_at = None      # at tiles of the previous (qq, kc)
            prev_avs = None     # av accumulators of the previous kc's qq
            pending = []        # deferred normalize steps from the last qq
            for qq in range(N_QQ):
                avs = [av_psum.tile([65, QQ_W], F32, tag="av", name=f"av_{qq}_{h}")
                       for h in range(2)]
                for kc in range(N_KC):
                    scs = [sc_psum.tile([128, QQ_W], F32, tag="sc",
                                        name=f"sc_{qq}_{kc}_{h}") for h in range(2)]
                    emit_sc(qq, kc, scs)
                    ats = emit_exps(qq, kc, scs)
                    if prev_at is not None:
                        emit_av(prev_avs, (kc - 1) % N_KC, prev_at)
                    if pending:
                        pending.pop(0)()
                    prev_at, prev_avs = ats, avs
                pending = list(norm_steps(qq, avs))
            # tail: trailing av of the last kc, then the last qq's normalize
            emit_av(prev_avs, N_KC - 1, prev_at)
            for step in pending:
                step()

        # ---- output projection ----
        with (
            tc.tile_pool(name="op_psum", bufs=3, space="PSUM") as op_psum,
            tc.tile_pool(name="op_sb", bufs=3) as op_sb,
        ):
            # earlier qq's outT is long finished; do the last qq's tiles last
            # so its normalize DMA chain hides behind the other 24 tiles
            nts = [nt for nt in range(N // 128) if nt < (N_QQ - 1) * (QQ_W // 128)]
            nts += [nt for nt in range(N // 128) if nt >= (N_QQ - 1) * (QQ_W // 128)]
            for nt in nts:
                po = op_psum.tile([128, D], F32, tag="po")
                nc.tensor.matmul(po, outT[:, 0, bass.ts(nt, 128)], wo_sb[:, 0, :],
                                 start=True, stop=False)
                nc.tensor.matmul(po, outT[:, 1, bass.ts(nt, 128)], wo_sb[:, 1, :],
                                 start=False, stop=True)
                ob = op_sb.tile([128, D], F32, tag="ob")
                nc.vector.tensor_copy(ob, po)
                nc.sync.dma_start(out=out[bass.ts(nt, 128), :], in_=ob)

    nc.compile()
    return nc


_NC_CACHE = None


def build_in_maps(x, Wq, Wk, Wv, Wo):
    bf = ml_dtypes.bfloat16
    x = np.asarray(x, np.float32)
    Wq, Wk, Wv, Wo = (np.asarray(a, np.float32) for a in (Wq, Wk, Wv, Wo))
    in_maps = []
    for c in range(8):
        b = c // 4
        h0 = 2 * (c % 4)
        xT = np.ascontiguousarray(x[b].T.astype(bf))
        wqT = np.ascontiguousarray(Wq[h0 * 64:(h0 + 2) * 64].T.astype(bf))
        wkT = np.ascontiguousarray(Wk[h0 * 64:(h0 + 2) * 64].T.astype(bf))
        wvT = np.ascontiguousarray(Wv[h0 * 64:(h0 + 2) * 64].T.astype(bf))
        woT = np.stack([np.ascontiguousarray(Wo[:, (h0 + h) * 64:(h0 + h + 1) * 64].T.astype(bf))
                        for h in range(2)])
        in_maps.append({"xT": xT, "wqT": wqT, "wkT": wkT, "wvT": wvT, "woT": woT})
    return in_maps


def kernel(x, Wq, Wk, Wv, Wo, bo):
    global _NC_CACHE
    bo = np.asarray(bo, np.float32)
    in_maps = build_in_maps(x, Wq, Wk, Wv, Wo)

    if _NC_CACHE is None:
        _NC_CACHE = build_bass()
    res = run_bass_kernel_spmd(_NC_CACHE, in_maps, list(range(8)))
    partials = [np.asarray(res.results[c]["out"], np.float32) for c in range(8)]

    out = np.empty((B, N, D), np.float32)
    for b in range(B):
        out[b] = partials[4 * b] + partials[4 * b + 1] + partials[4 * b + 2] + partials[4 * b + 3] + bo
    return out


if __name__ == "__main__":
    nc = build_bass()
    print("built ok")
